# revision 41
# baseline (speedup 1.0000x reference)
"""GATv2 3-layer GNN on 8 Trainium2 NeuronCores (Bass/Tile) — v5.

Key structure (per core):
  - Nodes are host-binned into 8*49=392 blocks of 128 slots with balanced
    in-degree per block.
  - The xl gather uses gpsimd dma_gather (one instruction per table-half
    per block, ~1us SWDGE each) instead of per-tile indirect DMAs
    (994ns fixed overhead each, 16 per block).  dma_gather indices are
    int16, so the 50176-row table is split in two halves; each block's
    edges are packed half-0-first into whole 128-slot tiles, padded with
    dummy row-0 gathers so every core runs identical shapes (SPMD).
  - Tables are bf16 (dma_gather needs 256B-multiple rows; also improves
    accuracy over fp8).  Layer 2 (D=64) pads table rows to 128 cols.
  - Layer 0: every core builds the FULL xl0 table locally from a
    pre-transposed bf16 copy of x (no AllGather for layer 0).
  - Layers 1,2: xl shards are exchanged with a 2-chunk AllGather
    (blocks 0..CHA-1 early, rest late) so most of the exchange hides
    behind phase-B work of the producing layer.
  - Phase B is software-pipelined: stage1 (mask build, xr-expand
    matmuls batched into one PSUM tile + one copy per half, dma_gather)
    runs LAG blocks ahead of stage2 (edge math, one-hot aggregation,
    node update + fused projection of the NEXT layer).
  - bias trick: the table holds x@Wl WITHOUT bias; bl is folded into
    xr's bias (v = xl'+xr' is unchanged) and into the output bias.
"""

import sys

if "/opt/trn_rl_repo" not in sys.path:
    sys.path.insert(0, "/opt/trn_rl_repo")

import numpy as np
import ml_dtypes

BF16 = ml_dtypes.bfloat16

NEG_SLOPE = 0.2
N_NODES = 50000
N_EDGES = 800000
N_GRAPHS = 64
IN_CH = 128
HIDDEN = 128
HEADS = 4
OUT_CH = 64
NCORES = 8


def make_cfg(n_nodes=N_NODES, n_graphs=N_GRAPHS, in_ch=IN_CH):
    npc = n_nodes // NCORES
    assert npc * NCORES == n_nodes
    nblk = (npc + 127) // 128
    np_pad = nblk * 128
    cha = max(1, (nblk * 4) // 5)  # early AG chunk (blocks [0, cha))
    trows = NCORES * np_pad
    return dict(
        N=n_nodes,
        G=n_graphs,
        NPC=npc,
        NP=np_pad,
        NBLK=nblk,
        CHA=cha,
        CHB=nblk - cha,
        GBLK=NCORES * nblk,
        TROWS=trows,
        HB1=16768,
        HB2=33536,
        IN_CH=in_ch,
        TS=None,  # per-block tile counts per table third (list of 3 lists)
        LAYERS=[
            (in_ch, HIDDEN, HEADS, HIDDEN // HEADS, True),
            (HIDDEN, HIDDEN, HEADS, HIDDEN // HEADS, True),
            (HIDDEN, OUT_CH, 1, OUT_CH, False),
        ],
    )


# ---------------------------------------------------------------- host prep
def _balanced_bins(deg, nbins, binsz):
    """Assign nodes to bins (each bin holds exactly binsz nodes) minimizing
    max total degree per bin.  Greedy: degree-desc, min-load non-full bin.
    Returns slot_of[node] = bin*binsz + position."""
    import heapq

    n = deg.shape[0]
    order = np.argsort(-deg, kind="stable")
    heap = [(0, b) for b in range(nbins)]
    heapq.heapify(heap)
    fill = np.zeros(nbins, np.int64)
    load = np.zeros(nbins, np.int64)
    slot_of = np.empty(n, np.int64)
    for nd in order:
        while True:
            l, b = heapq.heappop(heap)
            if fill[b] < binsz:
                break
        slot_of[nd] = b * binsz + fill[b]
        fill[b] += 1
        load[b] += deg[nd]
        if fill[b] < binsz:
            heapq.heappush(heap, (load[b], b))
    return slot_of, int(load.max())


def tabrow_of_slot(cfg, slot):
    """Map global slot id -> table row (2-chunk AllGather layout)."""
    NP, NBLK, CHA = cfg["NP"], cfg["NBLK"], cfg["CHA"]
    c = slot // NP
    loc = slot % NP
    b = loc // 128
    r = loc % 128
    rowsA = NCORES * CHA * 128
    return np.where(
        b < CHA,
        c * CHA * 128 + b * 128 + r,
        rowsA + c * (NBLK - CHA) * 128 + (b - CHA) * 128 + r,
    )


def _wrap16(lst):
    """dma_gather index layout: idx k -> [k%16, k//16], replicated x8."""
    n = lst.shape[0]
    assert n % 16 == 0
    w = lst.reshape(n // 16, 16).T  # [16, W]
    return np.tile(w, (8, 1)).astype(np.int16)  # [128, W]


def prep(cfg, x, edge_index, batch):
    NPC, NP, NBLK, G, CHA = cfg["NPC"], cfg["NP"], cfg["NBLK"], cfg["G"], cfg["CHA"]
    GBLK = cfg["GBLK"]
    Din = cfg["IN_CH"]
    src = np.asarray(edge_index[0], dtype=np.int64)
    dst = np.asarray(edge_index[1], dtype=np.int64)
    batch = np.asarray(batch, dtype=np.int64)
    x = np.asarray(x, dtype=np.float32)
    N = x.shape[0]

    deg = np.bincount(dst, minlength=N)
    slot_of, maxload = _balanced_bins(deg, GBLK, 128)

    node_of_slot = np.full(GBLK * 128, -1, np.int64)
    node_of_slot[slot_of] = np.arange(N)

    # permuted x, laid out in TABLE-ROW block order, transposed per block
    x_slot = np.zeros((GBLK * 128, Din), np.float32)
    valid = node_of_slot >= 0
    x_slot[valid] = x[node_of_slot[valid]]
    tabrow = np.asarray(tabrow_of_slot(cfg, np.arange(GBLK * 128)))
    x_tab = np.zeros_like(x_slot)
    x_tab[tabrow] = x_slot
    assert GBLK % 8 == 0
    xfullT = (
        x_tab.reshape(GBLK // 8, 8, 128, Din)
        .transpose(0, 3, 1, 2)
        .reshape((GBLK // 8) * Din, 8 * 128)
    ).astype(BF16)

    # edges
    sd = slot_of[dst]
    ss = slot_of[src]
    trow = np.asarray(tabrow_of_slot(cfg, ss))
    core_of = sd // NP
    dloc = sd % NP
    bloc = dloc // 128
    drow = dloc % 128

    # ---- pass 1: per (core, block) edge lists split by table third
    HB = [0, cfg["HB1"], cfg["HB2"], cfg["TROWS"]]
    third = np.digitize(trow, HB[1:3]).astype(np.int64)
    key = (core_of * NBLK + bloc) * 3 + third
    order = np.argsort(key, kind="stable")
    ks = key[order]
    tr_s = trow[order]
    ed_s = drow[order]
    bounds = np.searchsorted(ks, np.arange(NCORES * NBLK * 3 + 1))
    ed_rows = [[None] * NBLK for _ in range(NCORES)]  # [(rows_i, d_i) x3]
    for c in range(NCORES):
        for b in range(NBLK):
            k0i = (c * NBLK + b) * 3
            ed_rows[c][b] = [
                (tr_s[bounds[k0i + i] : bounds[k0i + i + 1]] - HB[i],
                 ed_s[bounds[k0i + i] : bounds[k0i + i + 1]])
                for i in range(3)]

    # ---- pass 2: shared per-block tile counts (max over cores)
    TS = [[1] * NBLK for _ in range(3)]
    for b in range(NBLK):
        for c in range(NCORES):
            for i in range(3):
                r_i, _ = ed_rows[c][b][i]
                TS[i][b] = max(TS[i][b], (len(r_i) + 127) // 128)
    assert max(max(t) for t in TS) <= 8, [max(t) for t in TS]
    cfg["TS"] = TS
    TMAXH = [max(t) for t in TS]
    TMAX = max(a + b + c_ for a, b, c_ in zip(*TS))
    cfg["TMAXH"], cfg["TMAX"] = TMAXH, TMAX
    WS = [t * 8 for t in TMAXH]

    # ---- pass 3: per-core arrays
    maps = []
    F8 = ml_dtypes.float8_e4m3
    ar128 = np.arange(128, dtype=np.int64)
    WTOT = sum(WS)
    for c in range(NCORES):
        idxg = np.zeros((NBLK * 128, WTOT), np.int16)
        mT_h = np.zeros((NBLK * 128, TMAX * 128), F8)
        mE_h = np.zeros((NBLK * 128, TMAX * 128), F8)
        for b in range(NBLK):
            ts = [TS[i][b] for i in range(3)]
            Tb = sum(ts)
            dv = np.full(Tb * 128, -1, np.int64)
            woff, soff = 0, 0
            for i in range(3):
                r_i, d_i = ed_rows[c][b][i]
                Li = np.zeros(ts[i] * 128, np.int64)
                Li[: len(r_i)] = r_i
                idxg[b * 128 : (b + 1) * 128, woff : woff + ts[i] * 8] = _wrap16(Li)
                dv[soff : soff + len(d_i)] = d_i
                woff += WS[i]
                soff += ts[i] * 128
            mT_h[b * 128 : (b + 1) * 128, : Tb * 128] = (
                dv[None, :] == ar128[:, None]).astype(F8)
            dc = dv.reshape(Tb, 128).T  # [128(p), Tb]
            mE_h[b * 128 : (b + 1) * 128, : Tb * 128] = (
                dc[:, :, None] == ar128[None, None, :]).reshape(128, Tb * 128).astype(F8)

        # own x^T blocks (for the xr projection pass), in own-block order
        own_tabrows = np.asarray(tabrow_of_slot(cfg, c * NP + np.arange(NP)))
        xownT = (
            x_tab[own_tabrows]
            .reshape(NBLK, 128, Din)
            .transpose(0, 2, 1)
            .reshape(NBLK * Din, 128)
        ).astype(BF16)

        # pool mask [NP, G] over own slots
        pm = np.zeros((NP, G), np.float32)
        own_nodes = node_of_slot[c * NP : (c + 1) * NP]
        vv = own_nodes >= 0
        pm[np.arange(NP)[vv], batch[own_nodes[vv]]] = 1.0

        maps.append(
            dict(
                xfullT=xfullT,
                xownT=xownT,
                idxg=idxg,
                mT_h=mT_h,
                mE_h=mE_h,
                pool_mask=pm.astype(BF16),
            )
        )

    counts = np.bincount(batch, minlength=G).astype(np.float32)
    return maps, counts


def prep_weights(cfg, inp):
    w = {}
    for l in range(3):
        Wl = np.asarray(inp[f"Wl{l}"], np.float32)
        bl = np.asarray(inp[f"bl{l}"], np.float32)
        Wr = np.asarray(inp[f"Wr{l}"], np.float32)
        br = np.asarray(inp[f"br{l}"], np.float32)
        bo = np.asarray(inp[f"bias{l}"], np.float32)
        D = Wl.shape[1]
        # table holds x@Wl (no bias); xr bias = bl+br; out bias += bl
        w[f"wcat{l}"] = np.concatenate([Wl, Wr], axis=1).astype(BF16)  # [Din,2D]
        w[f"bias_r{l}"] = np.broadcast_to((bl + br)[None, :], (128, D)).copy()
        w[f"bias_out{l}"] = np.broadcast_to((bo + bl)[None, :], (128, D)).copy()
    TMAX = cfg["TMAX"]
    for l in range(3):
        D = [HIDDEN, HIDDEN, OUT_CH][l]
        w[f"att{l}r"] = np.broadcast_to(
            np.asarray(inp[f"att{l}"], np.float32).reshape(1, 1, D), (128, TMAX, D)
        ).reshape(128, TMAX * D).astype(BF16)
    w["ident"] = np.eye(128, dtype=np.float32)
    return w


# ---------------------------------------------------------------- device build
def build(cfg):
    from concourse import bass, bacc, mybir
    import concourse.tile as tile
    from concourse.tile import add_dep_helper

    F32 = mybir.dt.float32
    BF = mybir.dt.bfloat16
    F8 = mybir.dt.float8e4
    I16 = mybir.dt.int16
    A = mybir.AluOpType
    ACTF = mybir.ActivationFunctionType

    NP, NBLK, TROWS, G = cfg["NP"], cfg["NBLK"], cfg["TROWS"], cfg["G"]
    CHA, CHB, GBLK = cfg["CHA"], cfg["CHB"], cfg["GBLK"]
    HB = [0, cfg["HB1"], cfg["HB2"], cfg["TROWS"]]
    TS, TMAX = cfg["TS"], cfg["TMAX"]
    TMAXH = cfg["TMAXH"]
    WS = [t * 8 for t in TMAXH]
    WTOT = sum(WS)
    WOFFS = [0, WS[0], WS[0] + WS[1]]
    Din0 = cfg["IN_CH"]
    LAYERS = cfg["LAYERS"]
    LAG = 2

    nc = bacc.Bacc(
        "TRN2",
        target_bir_lowering=False,
        debug=False,
        enable_asserts=False,
        num_devices=NCORES,
        num_swdge_queues=4,
    )

    ext = {}

    def ein(name, shape, dt):
        ext[name] = nc.dram_tensor(name, shape, dt, kind="ExternalInput").ap()
        return ext[name]

    xfullT = ein("xfullT", [(GBLK // 8) * Din0, 8 * 128], BF)
    xownT = ein("xownT", [NBLK * Din0, 128], BF)
    idxg_d = ein("idxg", [NBLK * 128, WTOT], I16)
    mT_d = ein("mT_h", [NBLK * 128, TMAX * 128], F8)
    mE_d = ein("mE_h", [NBLK * 128, TMAX * 128], F8)
    pool_mask = ein("pool_mask", [NP, G], BF)
    ident_d = ein("ident", [128, 128], F32)
    wcat_d, biasr_d, att_d, biasout_d = [], [], [], []
    for l, (Din, D, H, C, _) in enumerate(LAYERS):
        wcat_d.append(ein(f"wcat{l}", [Din, 2 * D], BF))
        biasr_d.append(ein(f"bias_r{l}", [128, D], F32))
        att_d.append(ein(f"att{l}r", [128, TMAX * D], BF))
        biasout_d.append(ein(f"bias_out{l}", [128, D], F32))

    pool_out = nc.dram_tensor("pool_out", [G, OUT_CH], F32, kind="ExternalOutput").ap()

    # internal DRAM: tables are bf16, 128 cols even for layer 2 (gather rows
    # must be 256B multiples)
    tabs = []
    ccA, ccB = [None] * 3, [None] * 3
    for l in range(3):
        tabs.append(
            nc.dram_tensor(
                f"tab{l}", [TROWS, 128], BF, kind="Internal", addr_space="Shared"
            ).ap()
        )
        if l >= 1:
            ccA[l] = nc.dram_tensor(f"ccA{l}", [CHA * 128, 128], BF, kind="Internal").ap()
            ccB[l] = nc.dram_tensor(f"ccB{l}", [CHB * 128, 128], BF, kind="Internal").ap()

    from contextlib import ExitStack

    with tile.TileContext(nc) as tc, ExitStack() as pools:
        const = pools.enter_context(tc.tile_pool(name="const", bufs=1))
        s1 = pools.enter_context(tc.tile_pool(name="s1", bufs=LAG + 2))
        s1b = pools.enter_context(tc.tile_pool(name="s1b", bufs=3))
        s2 = pools.enter_context(tc.tile_pool(name="s2", bufs=3))
        nodep = pools.enter_context(tc.tile_pool(name="nodep", bufs=3))
        # PSUM: 8 banks x 2KB.  vexp 4 banks, agg 1, pam 1, pat 1, pool 1.
        psum_exp = pools.enter_context(tc.tile_pool(name="psum_exp", bufs=1, space="PSUM"))
        psum_agg = pools.enter_context(tc.tile_pool(name="psum_agg", bufs=2, space="PSUM"))
        psum_pam = pools.enter_context(tc.tile_pool(name="psum_pam", bufs=2, space="PSUM"))
        psum_pat = pools.enter_context(tc.tile_pool(name="psum_pat", bufs=1, space="PSUM"))
        psum_pool = pools.enter_context(tc.tile_pool(name="psum_pool", bufs=1, space="PSUM"))

        # persistent SBUF: xr tables (double-buffered across layers)
        xr_sb = [
            nc.alloc_sbuf_tensor(f"xr_sb{k}", [128, NBLK, HIDDEN], BF).ap()
            for k in range(2)
        ]

        def const_tile(shape, dt, src_ap, tag):
            t = const.tile(shape, dt, tag=tag)
            nc.sync.dma_start(out=t[:], in_=src_ap)
            return t

        ident = const_tile([128, 128], F32, ident_d[:], "ident")
        wcat_s, biasr_s, att_s, biasout_s = [], [], [], []
        for l, (Din, D, H, C, _) in enumerate(LAYERS):
            wcat_s.append(const_tile([Din, 2 * D], BF, wcat_d[l][:], f"wc{l}"))
            biasr_s.append(const_tile([128, D], F32, biasr_d[l][:], f"br{l}"))
            att_s.append(const_tile([128, TMAX * D], BF, att_d[l][:], f"at{l}"))
            biasout_s.append(const_tile([128, D], F32, biasout_d[l][:], f"bo{l}"))

        # ============ layer 0: local full-table build + own xr pass
        D0 = LAYERS[0][1]
        tab0_writes = []
        GRP = 8
        assert GBLK % GRP == 0
        for gg in range(GBLK // GRP):
            xT8 = nodep.tile([Din0, GRP, 128], BF, tag="t0_xT")
            nc.sync.dma_start(
                out=xT8[:],
                in_=xfullT[gg * Din0 : (gg + 1) * Din0, :],
            )
            vps8 = psum_exp.tile([128, GRP, D0], F32, tag="vexp")
            for k in range(GRP):
                nc.tensor.matmul(
                    out=vps8[:, k, :], lhsT=xT8[:, k, :], rhs=wcat_s[0][:, :D0],
                    start=True, stop=True
                )
            xl8 = nodep.tile([128, GRP, D0], BF, tag="t0_xl")
            nc.scalar.copy(out=xl8[:], in_=vps8[:])
            wi = nc.sync.dma_start(
                out=tabs[0][gg * GRP * 128 : (gg + 1) * GRP * 128, :].rearrange(
                    "(g p) d -> p g d", g=GRP
                ),
                in_=xl8[:],
            )
            tab0_writes.append(wi)

        for b in range(NBLK):
            xT = nodep.tile([Din0, 128], BF, tag="own_xT")
            nc.sync.dma_start(out=xT[:], in_=xownT[b * Din0 : (b + 1) * Din0, :])
            pr_f = psum_pam.tile([128, 2 * HIDDEN], F32, tag="pa_mm")
            pr = pr_f[:, :D0]
            nc.tensor.matmul(
                out=pr, lhsT=xT[:], rhs=wcat_s[0][:, D0:], start=True, stop=True
            )
            nc.vector.tensor_tensor(
                out=xr_sb[0][:, b, :D0], in0=pr, in1=biasr_s[0][:], op=A.add
            )

        # per-third barrier proxies: third-h gathers only need table rows
        # [HB[h], HB[h+1]), i.e. the build groups covering those rows
        GROWS = GRP * 128
        barriers0 = []
        for h in range(3):
            g_lo = HB[h] // GROWS
            g_hi = (HB[h + 1] + GROWS - 1) // GROWS
            bar = nc.scalar.copy(out=ident[:1, h : h + 1], in_=ident[:1, h : h + 1])
            for wi in tab0_writes[g_lo:g_hi]:
                add_dep_helper(bar.ins, wi.ins, sync=True, reason=f"tab0 third{h}")
            barriers0.append(bar)

        # ============ layers
        ag_calls = {0: barriers0}  # per-layer: dep list (len 3 => per-third)

        for l, (Din, D, H, C, use_elu) in enumerate(LAYERS):
            HD = H + D
            xr_cur = xr_sb[l % 2]
            xr_nxt = xr_sb[(l + 1) % 2]
            gather_deps = ag_calls[l]
            if l < 2:
                pa_writesA, pa_writesB = [], []
            if l == 2:
                pool_ps = psum_pool.tile([G, OUT_CH], F32, tag="pool")

            state = {}

            def stage1(b, l=l, D=D, state=state,
                       xr_cur=xr_cur, gather_deps=gather_deps):
                ts = [TS[i][b] for i in range(3)]
                Tb = sum(ts)
                toffs = [0, ts[0], ts[0] + ts[1]]
                idxt = s1.tile([128, WTOT], I16, tag="idxt")
                nc.sync.dma_start(
                    out=idxt[:], in_=idxg_d[b * 128 : (b + 1) * 128, :]
                )
                # batched gathers FIRST (long DMA drain overlaps the rest of
                # stage1): dma_gather per table third, chunked to <=8 tiles
                # (1024 descs) -- the SWDGE ring holds 1024 descriptors
                g_all = s1.tile([128, Tb, 128], BF, tag="g")
                qn = b % 4
                for h in range(3):
                    tb_h, toff, ioff = ts[h], toffs[h], WOFFS[h]
                    roff, rend = HB[h], HB[h + 1]
                    if len(gather_deps) == 3:
                        deps_h = [gather_deps[h]]  # layer 0: per-third barriers
                    else:
                        # layers 1/2: third h needs agA if it has rows in
                        # [0, rowsA), agB if in [rowsA, TROWS)
                        rowsA = NCORES * CHA * 128
                        deps_h = []
                        if HB[h] < rowsA:
                            deps_h.append(gather_deps[0])
                        if HB[h + 1] > rowsA and len(gather_deps) > 1:
                            deps_h.append(gather_deps[1])
                    done = 0
                    while done < tb_h:
                        ch = min(8, tb_h - done)
                        gi = nc.gpsimd.dma_gather(
                            out_ap=g_all[:, toff + done : toff + done + ch, :],
                            in_ap=tabs[l][roff:rend, :],
                            idxs_ap=idxt[:, ioff + done * 8 : ioff + (done + ch) * 8],
                            num_idxs=ch * 128, num_idxs_reg=ch * 128,
                            elem_size=128, queue_num=qn)
                        for dep in deps_h:
                            add_dep_helper(gi.ins, dep.ins, sync=True,
                                           reason="gather after table ready")
                        done += ch
                        qn = (qn + 1) % 4
                # host-precomputed one-hot masks (fp8): mT for xr-expansion,
                # mE for the dst aggregation
                mT = s1b.tile([128, Tb, 128], F8, tag="mT")
                nc.sync.dma_start(
                    out=mT[:].rearrange("p t q -> p (t q)"),
                    in_=mT_d[b * 128 : (b + 1) * 128, : Tb * 128])
                mE = s1.tile([128, Tb, 128], F8, tag="mE")
                nc.sync.dma_start(
                    out=mE[:].rearrange("p t q -> p (t q)"),
                    in_=mE_d[b * 128 : (b + 1) * 128, : Tb * 128])
                # xr expansion per third into one PSUM tile + one copy
                v_all = s1.tile([128, Tb, D], BF, tag="v")
                for h in range(3):
                    tb, toff = ts[h], toffs[h]
                    vps = psum_exp.tile([128, max(max(TMAXH), GRP), D0], F32,
                                        tag="vexp")
                    for t in range(tb):
                        nc.tensor.matmul(
                            out=vps[:, t, :D], lhsT=mT[:, toff + t, :],
                            rhs=xr_cur[:, b, :D], start=True, stop=True)
                    nc.scalar.copy(
                        out=v_all[:, toff : toff + tb, :], in_=vps[:, :tb, :D])
                state[b] = (v_all, g_all, mE)

            def stage2(b, l=l, D=D, H=H, C=C, HD=HD, state=state,
                       xr_cur=xr_cur, xr_nxt=xr_nxt, use_elu=use_elu):
                Tb = sum(TS[i][b] for i in range(3))
                v_all, g_all, mE = state.pop(b)
                gD = g_all[:, :, :D]
                vs = s2.tile([128, Tb, D], BF, tag="vsum")
                nc.vector.tensor_tensor(
                    out=vs[:], in0=gD, in1=v_all[:], op=A.add)
                v4 = gD.rearrange("p t (h c) -> p t h c", h=H)
                vf = vs[:].rearrange("p t d -> p (t d)")
                l_all = s2.tile([128, Tb * D], BF, tag="lrelu")
                nc.vector.scalar_tensor_tensor(
                    out=l_all[:], in0=vf, scalar=NEG_SLOPE, in1=vf,
                    op0=A.mult, op1=A.max)
                p_all = s2.tile([128, Tb, H, C], BF, tag="patt")
                nc.vector.tensor_tensor(
                    out=p_all[:].rearrange("p t h c -> p (t h c)"), in0=l_all[:],
                    in1=att_s[l][:, : Tb * D], op=A.mult)
                lg = s2.tile([128, Tb, H], F32, tag="lg")
                nc.vector.tensor_reduce(
                    out=lg[:], in_=p_all[:], axis=mybir.AxisListType.X, op=A.add)
                e_t = s2.tile([128, Tb, H, 1], BF, tag="expv")
                nc.scalar.activation(out=e_t[:], in_=lg[:], func=ACTF.Exp)
                w_all = s2.tile([128, Tb, HD], BF, tag="wall")
                nc.scalar.copy(out=w_all[:, :, :H], in_=e_t[:])
                nc.vector.tensor_tensor(
                    out=w_all[:, :, H:].rearrange("p t (h c) -> p t h c", h=H),
                    in0=v4,
                    in1=e_t[:].to_broadcast([128, Tb, H, C]), op=A.mult)

                o_ps = psum_agg.tile([128, HD], F32, tag="agg")
                for t in range(Tb):
                    nc.tensor.matmul(
                        out=o_ps[:], lhsT=mE[:, t, :], rhs=w_all[:, t, :],
                        start=(t == 0), stop=(t == Tb - 1))

                dn = nodep.tile([128, H], F32, tag="dn")
                nc.vector.tensor_scalar(
                    out=dn[:], in0=o_ps[:, :H], scalar1=1e-30, scalar2=None, op0=A.add)
                rc = nodep.tile([128, H], F32, tag="rc")
                nc.vector.reciprocal(out=rc[:], in_=dn[:])
                onorm = nodep.tile([128, H, C], F32, tag="onorm")
                nc.vector.tensor_tensor(
                    out=onorm[:],
                    in0=o_ps[:, H:].rearrange("p (h c) -> p h c", h=H),
                    in1=rc[:].rearrange("p (h o) -> p h o", h=H).to_broadcast([128, H, C]),
                    op=A.mult)
                hb = nodep.tile([128, D], F32, tag="hb")
                nc.vector.tensor_tensor(
                    out=hb[:], in0=onorm[:].rearrange("p h c -> p (h c)"),
                    in1=biasout_s[l][:], op=A.add)
                if use_elu:
                    amax = nodep.tile([128, D], F32, tag="amax")
                    nc.vector.tensor_scalar(
                        out=amax[:], in0=hb[:], scalar1=0.0, scalar2=None, op0=A.max)
                    amin = nodep.tile([128, D], F32, tag="amin")
                    nc.vector.tensor_scalar(
                        out=amin[:], in0=hb[:], scalar1=0.0, scalar2=None, op0=A.min)
                    aexp = nodep.tile([128, D], F32, tag="aexp")
                    nc.scalar.activation(out=aexp[:], in_=amin[:], func=ACTF.Exp)
                    h_t = nodep.tile([128, D], F32, tag="h")
                    nc.vector.scalar_tensor_tensor(
                        out=h_t[:], in0=amax[:], scalar=-1.0, in1=aexp[:],
                        op0=A.add, op1=A.add)
                    # fused projection for layer l+1
                    Dn = LAYERS[l + 1][1]
                    tp = psum_pat.tile([128, 128], F32, tag="pa_tr")
                    nc.tensor.transpose(out=tp[:], in_=h_t[:], identity=ident[:])
                    hT = nodep.tile([128, 128], BF, tag="pa_hT")
                    nc.scalar.copy(out=hT[:], in_=tp[:])
                    pp_f = psum_pam.tile([128, 2 * HIDDEN], F32, tag="pa_mm")
                    pp = pp_f[:, : 2 * Dn]
                    nc.tensor.matmul(
                        out=pp, lhsT=hT[:], rhs=wcat_s[l + 1][:],
                        start=True, stop=True)
                    xl_t = nodep.tile([128, 128], BF, tag="pa_xl")
                    nc.scalar.copy(out=xl_t[:, :Dn], in_=pp[:, :Dn])
                    if Dn < 128:
                        nc.scalar.copy(out=xl_t[:, Dn:], in_=pp[:, :128 - Dn])
                    if b < CHA:
                        wi = nc.sync.dma_start(
                            out=ccA[l + 1][b * 128 : (b + 1) * 128, :], in_=xl_t[:])
                        pa_writesA.append(wi)
                    else:
                        wi = nc.sync.dma_start(
                            out=ccB[l + 1][(b - CHA) * 128 : (b - CHA + 1) * 128, :],
                            in_=xl_t[:])
                        pa_writesB.append(wi)
                    nc.vector.tensor_tensor(
                        out=xr_nxt[:, b, :Dn], in0=pp[:, Dn : 2 * Dn],
                        in1=biasr_s[l + 1][:], op=A.add)
                    if b == CHA - 1:
                        agA = nc.gpsimd.collective_compute(
                            "AllGather", A.bypass,
                            replica_groups=[list(range(NCORES))],
                            ins=[ccA[l + 1][:]],
                            outs=[tabs[l + 1][0 : NCORES * CHA * 128, :]],
                        )
                        for wi in pa_writesA:
                            add_dep_helper(agA.ins, wi.ins, sync=True, reason="agA")
                        ag_calls[l + 1] = [agA]
                    if b == NBLK - 1:
                        agB = nc.gpsimd.collective_compute(
                            "AllGather", A.bypass,
                            replica_groups=[list(range(NCORES))],
                            ins=[ccB[l + 1][:]],
                            outs=[tabs[l + 1][NCORES * CHA * 128 :, :]],
                        )
                        for wi in pa_writesB:
                            add_dep_helper(agB.ins, wi.ins, sync=True, reason="agB")
                        ag_calls[l + 1].append(agB)
                else:
                    h2b = nodep.tile([128, D], BF, tag="h2b")
                    nc.vector.tensor_scalar(
                        out=h2b[:], in0=hb[:], scalar1=0.0, scalar2=None, op0=A.add)
                    pm_t = nodep.tile([128, G], BF, tag="pmt")
                    nc.sync.dma_start(
                        out=pm_t[:], in_=pool_mask[b * 128 : (b + 1) * 128, :])
                    nc.tensor.matmul(
                        out=pool_ps[:], lhsT=pm_t[:], rhs=h2b[:],
                        start=(b == 0), stop=(b == NBLK - 1))

            for b in range(NBLK + LAG):
                if b < NBLK:
                    stage1(b)
                if b >= LAG:
                    stage2(b - LAG)

        pool_sb = nodep.tile([G, OUT_CH], F32, tag="poolsb")
        nc.scalar.copy(out=pool_sb[:], in_=pool_ps[:])
        nc.sync.dma_start(out=pool_out[:], in_=pool_sb[:])

    nc.compile()
    return nc


# ---------------------------------------------------------------- runner
_BUILD_CACHE = {}


def run(cfg, inp, trace=False):
    from concourse import bass_utils

    maps, counts = prep(cfg, np.asarray(inp["x"], np.float32), inp["edge_index"], inp["batch"])
    w = prep_weights(cfg, inp)
    for m in maps:
        m.update(w)

    key = (cfg["N"], cfg["G"], tuple(tuple(t) for t in cfg["TS"]))
    if key not in _BUILD_CACHE:
        _BUILD_CACHE[key] = build(cfg)
    nc = _BUILD_CACHE[key]

    res = bass_utils.run_bass_kernel_spmd(
        nc, maps, core_ids=list(range(NCORES)), trace=trace
    )
    total = np.zeros((cfg["G"], OUT_CH), np.float64)
    for k in range(NCORES):
        total += res.results[k]["pool_out"].astype(np.float64)
    out = (total / np.maximum(counts, 1.0)[:, None]).astype(np.float32)
    return out, res


def kernel(**inputs) -> np.ndarray:
    cfg = make_cfg()
    out, _ = run(cfg, inputs, trace=False)
    return out


# revision 42
# speedup vs baseline: 1.0135x; 1.0135x over previous
"""GATv2 3-layer GNN on 8 Trainium2 NeuronCores (Bass/Tile) — v5.

Key structure (per core):
  - Nodes are host-binned into 8*49=392 blocks of 128 slots with balanced
    in-degree per block.
  - The xl gather uses gpsimd dma_gather (one instruction per table-half
    per block, ~1us SWDGE each) instead of per-tile indirect DMAs
    (994ns fixed overhead each, 16 per block).  dma_gather indices are
    int16, so the 50176-row table is split in two halves; each block's
    edges are packed half-0-first into whole 128-slot tiles, padded with
    dummy row-0 gathers so every core runs identical shapes (SPMD).
  - Tables are bf16 (dma_gather needs 256B-multiple rows; also improves
    accuracy over fp8).  Layer 2 (D=64) pads table rows to 128 cols.
  - Layer 0: every core builds the FULL xl0 table locally from a
    pre-transposed bf16 copy of x (no AllGather for layer 0).
  - Layers 1,2: xl shards are exchanged with a 2-chunk AllGather
    (blocks 0..CHA-1 early, rest late) so most of the exchange hides
    behind phase-B work of the producing layer.
  - Phase B is software-pipelined: stage1 (mask build, xr-expand
    matmuls batched into one PSUM tile + one copy per half, dma_gather)
    runs LAG blocks ahead of stage2 (edge math, one-hot aggregation,
    node update + fused projection of the NEXT layer).
  - bias trick: the table holds x@Wl WITHOUT bias; bl is folded into
    xr's bias (v = xl'+xr' is unchanged) and into the output bias.
"""

import sys

if "/opt/trn_rl_repo" not in sys.path:
    sys.path.insert(0, "/opt/trn_rl_repo")

import numpy as np
import ml_dtypes

BF16 = ml_dtypes.bfloat16

NEG_SLOPE = 0.2
N_NODES = 50000
N_EDGES = 800000
N_GRAPHS = 64
IN_CH = 128
HIDDEN = 128
HEADS = 4
OUT_CH = 64
NCORES = 8


def make_cfg(n_nodes=N_NODES, n_graphs=N_GRAPHS, in_ch=IN_CH):
    npc = n_nodes // NCORES
    assert npc * NCORES == n_nodes
    nblk = (npc + 127) // 128
    np_pad = nblk * 128
    cha = max(1, (nblk * 4) // 5)  # early AG chunk (blocks [0, cha))
    trows = NCORES * np_pad
    return dict(
        N=n_nodes,
        G=n_graphs,
        NPC=npc,
        NP=np_pad,
        NBLK=nblk,
        CHA=cha,
        CHB=nblk - cha,
        GBLK=NCORES * nblk,
        TROWS=trows,
        HB1=16768,
        HB2=33536,
        IN_CH=in_ch,
        TS=None,  # per-block tile counts per table third (list of 3 lists)
        LAYERS=[
            (in_ch, HIDDEN, HEADS, HIDDEN // HEADS, True),
            (HIDDEN, HIDDEN, HEADS, HIDDEN // HEADS, True),
            (HIDDEN, OUT_CH, 1, OUT_CH, False),
        ],
    )


# ---------------------------------------------------------------- host prep
def _balanced_bins(deg, nbins, binsz):
    """Assign nodes to bins (each bin holds exactly binsz nodes) minimizing
    max total degree per bin.  Greedy: degree-desc, min-load non-full bin.
    Returns slot_of[node] = bin*binsz + position."""
    import heapq

    n = deg.shape[0]
    order = np.argsort(-deg, kind="stable")
    heap = [(0, b) for b in range(nbins)]
    heapq.heapify(heap)
    fill = np.zeros(nbins, np.int64)
    load = np.zeros(nbins, np.int64)
    slot_of = np.empty(n, np.int64)
    for nd in order:
        while True:
            l, b = heapq.heappop(heap)
            if fill[b] < binsz:
                break
        slot_of[nd] = b * binsz + fill[b]
        fill[b] += 1
        load[b] += deg[nd]
        if fill[b] < binsz:
            heapq.heappush(heap, (load[b], b))
    return slot_of, int(load.max())


def tabrow_of_slot(cfg, slot):
    """Map global slot id -> table row (2-chunk AllGather layout)."""
    NP, NBLK, CHA = cfg["NP"], cfg["NBLK"], cfg["CHA"]
    c = slot // NP
    loc = slot % NP
    b = loc // 128
    r = loc % 128
    rowsA = NCORES * CHA * 128
    return np.where(
        b < CHA,
        c * CHA * 128 + b * 128 + r,
        rowsA + c * (NBLK - CHA) * 128 + (b - CHA) * 128 + r,
    )


def _wrap16(lst):
    """dma_gather index layout: idx k -> [k%16, k//16], replicated x8."""
    n = lst.shape[0]
    assert n % 16 == 0
    w = lst.reshape(n // 16, 16).T  # [16, W]
    return np.tile(w, (8, 1)).astype(np.int16)  # [128, W]


def prep(cfg, x, edge_index, batch):
    NPC, NP, NBLK, G, CHA = cfg["NPC"], cfg["NP"], cfg["NBLK"], cfg["G"], cfg["CHA"]
    GBLK = cfg["GBLK"]
    Din = cfg["IN_CH"]
    src = np.asarray(edge_index[0], dtype=np.int64)
    dst = np.asarray(edge_index[1], dtype=np.int64)
    batch = np.asarray(batch, dtype=np.int64)
    x = np.asarray(x, dtype=np.float32)
    N = x.shape[0]

    deg = np.bincount(dst, minlength=N)
    slot_of, maxload = _balanced_bins(deg, GBLK, 128)

    node_of_slot = np.full(GBLK * 128, -1, np.int64)
    node_of_slot[slot_of] = np.arange(N)

    # permuted x, laid out in TABLE-ROW block order, transposed per block
    x_slot = np.zeros((GBLK * 128, Din), np.float32)
    valid = node_of_slot >= 0
    x_slot[valid] = x[node_of_slot[valid]]
    tabrow = np.asarray(tabrow_of_slot(cfg, np.arange(GBLK * 128)))
    x_tab = np.zeros_like(x_slot)
    x_tab[tabrow] = x_slot
    assert GBLK % 8 == 0
    xfullT = (
        x_tab.reshape(GBLK // 8, 8, 128, Din)
        .transpose(0, 3, 1, 2)
        .reshape((GBLK // 8) * Din, 8 * 128)
    ).astype(BF16)

    # edges
    sd = slot_of[dst]
    ss = slot_of[src]
    trow = np.asarray(tabrow_of_slot(cfg, ss))
    core_of = sd // NP
    dloc = sd % NP
    bloc = dloc // 128
    drow = dloc % 128

    # ---- pass 1: per (core, block) edge lists split by table third
    HB = [0, cfg["HB1"], cfg["HB2"], cfg["TROWS"]]
    third = np.digitize(trow, HB[1:3]).astype(np.int64)
    key = (core_of * NBLK + bloc) * 3 + third
    order = np.argsort(key, kind="stable")
    ks = key[order]
    tr_s = trow[order]
    ed_s = drow[order]
    bounds = np.searchsorted(ks, np.arange(NCORES * NBLK * 3 + 1))
    ed_rows = [[None] * NBLK for _ in range(NCORES)]  # [(rows_i, d_i) x3]
    for c in range(NCORES):
        for b in range(NBLK):
            k0i = (c * NBLK + b) * 3
            ed_rows[c][b] = [
                (tr_s[bounds[k0i + i] : bounds[k0i + i + 1]] - HB[i],
                 ed_s[bounds[k0i + i] : bounds[k0i + i + 1]])
                for i in range(3)]

    # ---- pass 2: shared per-block tile counts (max over cores)
    TS = [[1] * NBLK for _ in range(3)]
    for b in range(NBLK):
        for c in range(NCORES):
            for i in range(3):
                r_i, _ = ed_rows[c][b][i]
                TS[i][b] = max(TS[i][b], (len(r_i) + 127) // 128)
    assert max(max(t) for t in TS) <= 8, [max(t) for t in TS]
    cfg["TS"] = TS
    TMAXH = [max(t) for t in TS]
    TMAX = max(a + b + c_ for a, b, c_ in zip(*TS))
    cfg["TMAXH"], cfg["TMAX"] = TMAXH, TMAX
    WS = [t * 8 for t in TMAXH]

    # ---- pass 3: per-core arrays
    maps = []
    F8 = ml_dtypes.float8_e4m3
    ar128 = np.arange(128, dtype=np.int64)
    WTOT = sum(WS)
    for c in range(NCORES):
        idxg = np.zeros((NBLK * 128, WTOT), np.int16)
        mT_h = np.zeros((NBLK * 128, TMAX * 128), F8)
        mE_h = np.zeros((NBLK * 128, TMAX * 128), F8)
        for b in range(NBLK):
            ts = [TS[i][b] for i in range(3)]
            Tb = sum(ts)
            dv = np.full(Tb * 128, -1, np.int64)
            woff, soff = 0, 0
            for i in range(3):
                r_i, d_i = ed_rows[c][b][i]
                Li = np.zeros(ts[i] * 128, np.int64)
                Li[: len(r_i)] = r_i
                idxg[b * 128 : (b + 1) * 128, woff : woff + ts[i] * 8] = _wrap16(Li)
                dv[soff : soff + len(d_i)] = d_i
                woff += WS[i]
                soff += ts[i] * 128
            mT_h[b * 128 : (b + 1) * 128, : Tb * 128] = (
                dv[None, :] == ar128[:, None]).astype(F8)
            dc = dv.reshape(Tb, 128).T  # [128(p), Tb]
            mE_h[b * 128 : (b + 1) * 128, : Tb * 128] = (
                dc[:, :, None] == ar128[None, None, :]).reshape(128, Tb * 128).astype(F8)

        # own x^T blocks (for the xr projection pass), in own-block order
        own_tabrows = np.asarray(tabrow_of_slot(cfg, c * NP + np.arange(NP)))
        xownT = (
            x_tab[own_tabrows]
            .reshape(NBLK, 128, Din)
            .transpose(0, 2, 1)
            .reshape(NBLK * Din, 128)
        ).astype(BF16)

        # pool mask [NP, G] over own slots
        pm = np.zeros((NP, G), np.float32)
        own_nodes = node_of_slot[c * NP : (c + 1) * NP]
        vv = own_nodes >= 0
        pm[np.arange(NP)[vv], batch[own_nodes[vv]]] = 1.0

        maps.append(
            dict(
                xfullT=xfullT,
                xownT=xownT,
                idxg=idxg,
                mT_h=mT_h,
                mE_h=mE_h,
                pool_mask=pm.astype(BF16),
            )
        )

    counts = np.bincount(batch, minlength=G).astype(np.float32)
    return maps, counts


def prep_weights(cfg, inp):
    w = {}
    for l in range(3):
        Wl = np.asarray(inp[f"Wl{l}"], np.float32)
        bl = np.asarray(inp[f"bl{l}"], np.float32)
        Wr = np.asarray(inp[f"Wr{l}"], np.float32)
        br = np.asarray(inp[f"br{l}"], np.float32)
        bo = np.asarray(inp[f"bias{l}"], np.float32)
        D = Wl.shape[1]
        # table holds x@Wl (no bias); xr bias = bl+br; out bias += bl
        w[f"wcat{l}"] = np.concatenate([Wl, Wr], axis=1).astype(BF16)  # [Din,2D]
        w[f"bias_r{l}"] = np.broadcast_to((bl + br)[None, :], (128, D)).copy()
        w[f"bias_out{l}"] = np.broadcast_to((bo + bl)[None, :], (128, D)).copy()
    TMAX = cfg["TMAX"]
    for l in range(3):
        D = [HIDDEN, HIDDEN, OUT_CH][l]
        w[f"att{l}r"] = np.broadcast_to(
            np.asarray(inp[f"att{l}"], np.float32).reshape(1, 1, D), (128, TMAX, D)
        ).reshape(128, TMAX * D).astype(BF16)
    w["ident"] = np.eye(128, dtype=np.float32)
    return w


# ---------------------------------------------------------------- device build
def build(cfg):
    from concourse import bass, bacc, mybir
    import concourse.tile as tile
    from concourse.tile import add_dep_helper

    F32 = mybir.dt.float32
    BF = mybir.dt.bfloat16
    F8 = mybir.dt.float8e4
    I16 = mybir.dt.int16
    A = mybir.AluOpType
    ACTF = mybir.ActivationFunctionType

    NP, NBLK, TROWS, G = cfg["NP"], cfg["NBLK"], cfg["TROWS"], cfg["G"]
    CHA, CHB, GBLK = cfg["CHA"], cfg["CHB"], cfg["GBLK"]
    HB = [0, cfg["HB1"], cfg["HB2"], cfg["TROWS"]]
    TS, TMAX = cfg["TS"], cfg["TMAX"]
    TMAXH = cfg["TMAXH"]
    WS = [t * 8 for t in TMAXH]
    WTOT = sum(WS)
    WOFFS = [0, WS[0], WS[0] + WS[1]]
    Din0 = cfg["IN_CH"]
    LAYERS = cfg["LAYERS"]
    LAG = 2

    nc = bacc.Bacc(
        "TRN2",
        target_bir_lowering=False,
        debug=False,
        enable_asserts=False,
        num_devices=NCORES,
        num_swdge_queues=4,
    )

    ext = {}

    def ein(name, shape, dt):
        ext[name] = nc.dram_tensor(name, shape, dt, kind="ExternalInput").ap()
        return ext[name]

    xfullT = ein("xfullT", [(GBLK // 8) * Din0, 8 * 128], BF)
    xownT = ein("xownT", [NBLK * Din0, 128], BF)
    idxg_d = ein("idxg", [NBLK * 128, WTOT], I16)
    mT_d = ein("mT_h", [NBLK * 128, TMAX * 128], F8)
    mE_d = ein("mE_h", [NBLK * 128, TMAX * 128], F8)
    pool_mask = ein("pool_mask", [NP, G], BF)
    ident_d = ein("ident", [128, 128], F32)
    wcat_d, biasr_d, att_d, biasout_d = [], [], [], []
    for l, (Din, D, H, C, _) in enumerate(LAYERS):
        wcat_d.append(ein(f"wcat{l}", [Din, 2 * D], BF))
        biasr_d.append(ein(f"bias_r{l}", [128, D], F32))
        att_d.append(ein(f"att{l}r", [128, TMAX * D], BF))
        biasout_d.append(ein(f"bias_out{l}", [128, D], F32))

    pool_out = nc.dram_tensor("pool_out", [G, OUT_CH], F32, kind="ExternalOutput").ap()

    # internal DRAM: tables are bf16, 128 cols even for layer 2 (gather rows
    # must be 256B multiples)
    tabs = []
    ccA, ccB = [None] * 3, [None] * 3
    for l in range(3):
        tabs.append(
            nc.dram_tensor(
                f"tab{l}", [TROWS, 128], BF, kind="Internal", addr_space="Shared"
            ).ap()
        )
        if l >= 1:
            ccA[l] = nc.dram_tensor(f"ccA{l}", [CHA * 128, 128], BF, kind="Internal").ap()
            ccB[l] = nc.dram_tensor(f"ccB{l}", [CHB * 128, 128], BF, kind="Internal").ap()

    from contextlib import ExitStack

    with tile.TileContext(nc) as tc, ExitStack() as pools:
        const = pools.enter_context(tc.tile_pool(name="const", bufs=1))
        s1 = pools.enter_context(tc.tile_pool(name="s1", bufs=LAG + 2))
        s1b = pools.enter_context(tc.tile_pool(name="s1b", bufs=3))
        s2 = pools.enter_context(tc.tile_pool(name="s2", bufs=3))
        nodep = pools.enter_context(tc.tile_pool(name="nodep", bufs=3))
        # PSUM: 8 banks x 2KB.  vexp 4 banks, agg 1, pam 1, pat 1, pool 1.
        psum_exp = pools.enter_context(tc.tile_pool(name="psum_exp", bufs=1, space="PSUM"))
        psum_agg = pools.enter_context(tc.tile_pool(name="psum_agg", bufs=2, space="PSUM"))
        psum_pam = pools.enter_context(tc.tile_pool(name="psum_pam", bufs=2, space="PSUM"))
        psum_pat = pools.enter_context(tc.tile_pool(name="psum_pat", bufs=1, space="PSUM"))
        psum_pool = pools.enter_context(tc.tile_pool(name="psum_pool", bufs=1, space="PSUM"))

        # persistent SBUF: xr tables (double-buffered across layers)
        xr_sb = [
            nc.alloc_sbuf_tensor(f"xr_sb{k}", [128, NBLK, HIDDEN], BF).ap()
            for k in range(2)
        ]

        def const_tile(shape, dt, src_ap, tag):
            t = const.tile(shape, dt, tag=tag)
            nc.sync.dma_start(out=t[:], in_=src_ap)
            return t

        ident = const_tile([128, 128], F32, ident_d[:], "ident")
        wcat_s, biasr_s, att_s, biasout_s = [], [], [], []
        for l, (Din, D, H, C, _) in enumerate(LAYERS):
            wcat_s.append(const_tile([Din, 2 * D], BF, wcat_d[l][:], f"wc{l}"))
            biasr_s.append(const_tile([128, D], F32, biasr_d[l][:], f"br{l}"))
            att_s.append(const_tile([128, TMAX * D], BF, att_d[l][:], f"at{l}"))
            biasout_s.append(const_tile([128, D], F32, biasout_d[l][:], f"bo{l}"))

        # ============ layer 0: local full-table build + own xr pass
        D0 = LAYERS[0][1]
        tab0_writes = []
        GRP = 8
        assert GBLK % GRP == 0
        for gg in range(GBLK // GRP):
            xT8 = nodep.tile([Din0, GRP, 128], BF, tag="t0_xT")
            nc.sync.dma_start(
                out=xT8[:],
                in_=xfullT[gg * Din0 : (gg + 1) * Din0, :],
            )
            vps8 = psum_exp.tile([128, GRP, D0], F32, tag="vexp")
            for k in range(GRP):
                nc.tensor.matmul(
                    out=vps8[:, k, :], lhsT=xT8[:, k, :], rhs=wcat_s[0][:, :D0],
                    start=True, stop=True
                )
            xl8 = nodep.tile([128, GRP, D0], BF, tag="t0_xl")
            nc.scalar.copy(out=xl8[:], in_=vps8[:])
            wi = nc.sync.dma_start(
                out=tabs[0][gg * GRP * 128 : (gg + 1) * GRP * 128, :].rearrange(
                    "(g p) d -> p g d", g=GRP
                ),
                in_=xl8[:],
            )
            tab0_writes.append(wi)

        for b in range(NBLK):
            xT = nodep.tile([Din0, 128], BF, tag="own_xT")
            nc.sync.dma_start(out=xT[:], in_=xownT[b * Din0 : (b + 1) * Din0, :])
            pr_f = psum_pam.tile([128, 2 * HIDDEN], F32, tag="pa_mm")
            pr = pr_f[:, :D0]
            nc.tensor.matmul(
                out=pr, lhsT=xT[:], rhs=wcat_s[0][:, D0:], start=True, stop=True
            )
            nc.vector.tensor_tensor(
                out=xr_sb[0][:, b, :D0], in0=pr, in1=biasr_s[0][:], op=A.add
            )

        # per-third barrier proxies: third-h gathers only need table rows
        # [HB[h], HB[h+1]), i.e. the build groups covering those rows
        GROWS = GRP * 128
        barriers0 = []
        for h in range(3):
            g_lo = HB[h] // GROWS
            g_hi = (HB[h + 1] + GROWS - 1) // GROWS
            bar = nc.scalar.copy(out=ident[:1, h : h + 1], in_=ident[:1, h : h + 1])
            for wi in tab0_writes[g_lo:g_hi]:
                add_dep_helper(bar.ins, wi.ins, sync=True, reason=f"tab0 third{h}")
            barriers0.append(bar)

        # ============ layers
        ag_calls = {0: barriers0}  # per-layer: dep list (len 3 => per-third)

        for l, (Din, D, H, C, use_elu) in enumerate(LAYERS):
            HD = H + D
            xr_cur = xr_sb[l % 2]
            xr_nxt = xr_sb[(l + 1) % 2]
            gather_deps = ag_calls[l]
            if l < 2:
                pa_writesA, pa_writesB = [], []
            if l == 2:
                pool_ps = psum_pool.tile([G, OUT_CH], F32, tag="pool")

            state = {}

            def stage1(b, l=l, D=D, state=state,
                       xr_cur=xr_cur, gather_deps=gather_deps):
                ts = [TS[i][b] for i in range(3)]
                Tb = sum(ts)
                toffs = [0, ts[0], ts[0] + ts[1]]
                idxt = s1.tile([128, WTOT], I16, tag="idxt")
                nc.sync.dma_start(
                    out=idxt[:], in_=idxg_d[b * 128 : (b + 1) * 128, :]
                )
                # batched gathers FIRST (long DMA drain overlaps the rest of
                # stage1): dma_gather per table third, chunked to <=8 tiles
                # (1024 descs) -- the SWDGE ring holds 1024 descriptors
                g_all = s1.tile([128, Tb, 128], BF, tag="g")
                qn = b % 4
                for h in range(3):
                    tb_h, toff, ioff = ts[h], toffs[h], WOFFS[h]
                    roff, rend = HB[h], HB[h + 1]
                    deps_h = (
                        [gather_deps[h]] if len(gather_deps) == 3 else gather_deps)
                    done = 0
                    while done < tb_h:
                        ch = min(8, tb_h - done)
                        gi = nc.gpsimd.dma_gather(
                            out_ap=g_all[:, toff + done : toff + done + ch, :],
                            in_ap=tabs[l][roff:rend, :],
                            idxs_ap=idxt[:, ioff + done * 8 : ioff + (done + ch) * 8],
                            num_idxs=ch * 128, num_idxs_reg=ch * 128,
                            elem_size=128, queue_num=qn)
                        for dep in deps_h:
                            add_dep_helper(gi.ins, dep.ins, sync=True,
                                           reason="gather after table ready")
                        done += ch
                        qn = (qn + 1) % 4
                # host-precomputed one-hot masks (fp8): mT for xr-expansion,
                # mE for the dst aggregation
                mT = s1b.tile([128, Tb, 128], F8, tag="mT")
                nc.sync.dma_start(
                    out=mT[:].rearrange("p t q -> p (t q)"),
                    in_=mT_d[b * 128 : (b + 1) * 128, : Tb * 128])
                mE = s1.tile([128, Tb, 128], F8, tag="mE")
                nc.sync.dma_start(
                    out=mE[:].rearrange("p t q -> p (t q)"),
                    in_=mE_d[b * 128 : (b + 1) * 128, : Tb * 128])
                # xr expansion per third into one PSUM tile + one copy
                v_all = s1.tile([128, Tb, D], BF, tag="v")
                for h in range(3):
                    tb, toff = ts[h], toffs[h]
                    vps = psum_exp.tile([128, max(max(TMAXH), GRP), D0], F32,
                                        tag="vexp")
                    for t in range(tb):
                        nc.tensor.matmul(
                            out=vps[:, t, :D], lhsT=mT[:, toff + t, :],
                            rhs=xr_cur[:, b, :D], start=True, stop=True)
                    nc.scalar.copy(
                        out=v_all[:, toff : toff + tb, :], in_=vps[:, :tb, :D])
                state[b] = (v_all, g_all, mE)

            def stage2(b, l=l, D=D, H=H, C=C, HD=HD, state=state,
                       xr_cur=xr_cur, xr_nxt=xr_nxt, use_elu=use_elu):
                Tb = sum(TS[i][b] for i in range(3))
                v_all, g_all, mE = state.pop(b)
                gD = g_all[:, :, :D]
                vs = s2.tile([128, Tb, D], BF, tag="vsum")
                nc.vector.tensor_tensor(
                    out=vs[:], in0=gD, in1=v_all[:], op=A.add)
                v4 = gD.rearrange("p t (h c) -> p t h c", h=H)
                vf = vs[:].rearrange("p t d -> p (t d)")
                l_all = s2.tile([128, Tb * D], BF, tag="lrelu")
                nc.vector.scalar_tensor_tensor(
                    out=l_all[:], in0=vf, scalar=NEG_SLOPE, in1=vf,
                    op0=A.mult, op1=A.max)
                p_all = s2.tile([128, Tb, H, C], BF, tag="patt")
                nc.vector.tensor_tensor(
                    out=p_all[:].rearrange("p t h c -> p (t h c)"), in0=l_all[:],
                    in1=att_s[l][:, : Tb * D], op=A.mult)
                lg = s2.tile([128, Tb, H], F32, tag="lg")
                nc.vector.tensor_reduce(
                    out=lg[:], in_=p_all[:], axis=mybir.AxisListType.X, op=A.add)
                e_t = s2.tile([128, Tb, H, 1], BF, tag="expv")
                nc.scalar.activation(out=e_t[:], in_=lg[:], func=ACTF.Exp)
                w_all = s2.tile([128, Tb, HD], BF, tag="wall")
                nc.scalar.copy(out=w_all[:, :, :H], in_=e_t[:])
                nc.vector.tensor_tensor(
                    out=w_all[:, :, H:].rearrange("p t (h c) -> p t h c", h=H),
                    in0=v4,
                    in1=e_t[:].to_broadcast([128, Tb, H, C]), op=A.mult)

                o_ps = psum_agg.tile([128, HD], F32, tag="agg")
                for t in range(Tb):
                    nc.tensor.matmul(
                        out=o_ps[:], lhsT=mE[:, t, :], rhs=w_all[:, t, :],
                        start=(t == 0), stop=(t == Tb - 1))

                dn = nodep.tile([128, H], F32, tag="dn")
                nc.vector.tensor_scalar(
                    out=dn[:], in0=o_ps[:, :H], scalar1=1e-30, scalar2=None, op0=A.add)
                rc = nodep.tile([128, H], F32, tag="rc")
                nc.vector.reciprocal(out=rc[:], in_=dn[:])
                onorm = nodep.tile([128, H, C], F32, tag="onorm")
                nc.vector.tensor_tensor(
                    out=onorm[:],
                    in0=o_ps[:, H:].rearrange("p (h c) -> p h c", h=H),
                    in1=rc[:].rearrange("p (h o) -> p h o", h=H).to_broadcast([128, H, C]),
                    op=A.mult)
                hb = nodep.tile([128, D], F32, tag="hb")
                nc.vector.tensor_tensor(
                    out=hb[:], in0=onorm[:].rearrange("p h c -> p (h c)"),
                    in1=biasout_s[l][:], op=A.add)
                if use_elu:
                    amax = nodep.tile([128, D], F32, tag="amax")
                    nc.vector.tensor_scalar(
                        out=amax[:], in0=hb[:], scalar1=0.0, scalar2=None, op0=A.max)
                    amin = nodep.tile([128, D], F32, tag="amin")
                    nc.vector.tensor_scalar(
                        out=amin[:], in0=hb[:], scalar1=0.0, scalar2=None, op0=A.min)
                    aexp = nodep.tile([128, D], F32, tag="aexp")
                    nc.scalar.activation(out=aexp[:], in_=amin[:], func=ACTF.Exp)
                    h_t = nodep.tile([128, D], F32, tag="h")
                    nc.vector.scalar_tensor_tensor(
                        out=h_t[:], in0=amax[:], scalar=-1.0, in1=aexp[:],
                        op0=A.add, op1=A.add)
                    # fused projection for layer l+1
                    Dn = LAYERS[l + 1][1]
                    tp = psum_pat.tile([128, 128], F32, tag="pa_tr")
                    nc.tensor.transpose(out=tp[:], in_=h_t[:], identity=ident[:])
                    hT = nodep.tile([128, 128], BF, tag="pa_hT")
                    nc.scalar.copy(out=hT[:], in_=tp[:])
                    pp_f = psum_pam.tile([128, 2 * HIDDEN], F32, tag="pa_mm")
                    pp = pp_f[:, : 2 * Dn]
                    nc.tensor.matmul(
                        out=pp, lhsT=hT[:], rhs=wcat_s[l + 1][:],
                        start=True, stop=True)
                    xl_t = nodep.tile([128, 128], BF, tag="pa_xl")
                    nc.scalar.copy(out=xl_t[:, :Dn], in_=pp[:, :Dn])
                    if Dn < 128:
                        nc.scalar.copy(out=xl_t[:, Dn:], in_=pp[:, :128 - Dn])
                    if b < CHA:
                        wi = nc.sync.dma_start(
                            out=ccA[l + 1][b * 128 : (b + 1) * 128, :], in_=xl_t[:])
                        pa_writesA.append(wi)
                    else:
                        wi = nc.sync.dma_start(
                            out=ccB[l + 1][(b - CHA) * 128 : (b - CHA + 1) * 128, :],
                            in_=xl_t[:])
                        pa_writesB.append(wi)
                    nc.vector.tensor_tensor(
                        out=xr_nxt[:, b, :Dn], in0=pp[:, Dn : 2 * Dn],
                        in1=biasr_s[l + 1][:], op=A.add)
                    if b == CHA - 1:
                        agA = nc.gpsimd.collective_compute(
                            "AllGather", A.bypass,
                            replica_groups=[list(range(NCORES))],
                            ins=[ccA[l + 1][:]],
                            outs=[tabs[l + 1][0 : NCORES * CHA * 128, :]],
                        )
                        for wi in pa_writesA:
                            add_dep_helper(agA.ins, wi.ins, sync=True, reason="agA")
                        ag_calls[l + 1] = [agA]
                    if b == NBLK - 1:
                        agB = nc.gpsimd.collective_compute(
                            "AllGather", A.bypass,
                            replica_groups=[list(range(NCORES))],
                            ins=[ccB[l + 1][:]],
                            outs=[tabs[l + 1][NCORES * CHA * 128 :, :]],
                        )
                        for wi in pa_writesB:
                            add_dep_helper(agB.ins, wi.ins, sync=True, reason="agB")
                        ag_calls[l + 1].append(agB)
                else:
                    h2b = nodep.tile([128, D], BF, tag="h2b")
                    nc.vector.tensor_scalar(
                        out=h2b[:], in0=hb[:], scalar1=0.0, scalar2=None, op0=A.add)
                    pm_t = nodep.tile([128, G], BF, tag="pmt")
                    nc.sync.dma_start(
                        out=pm_t[:], in_=pool_mask[b * 128 : (b + 1) * 128, :])
                    nc.tensor.matmul(
                        out=pool_ps[:], lhsT=pm_t[:], rhs=h2b[:],
                        start=(b == 0), stop=(b == NBLK - 1))

            for b in range(NBLK + LAG):
                if b < NBLK:
                    stage1(b)
                if b >= LAG:
                    stage2(b - LAG)

        pool_sb = nodep.tile([G, OUT_CH], F32, tag="poolsb")
        nc.scalar.copy(out=pool_sb[:], in_=pool_ps[:])
        nc.sync.dma_start(out=pool_out[:], in_=pool_sb[:])

    nc.compile()
    return nc


# ---------------------------------------------------------------- runner
_BUILD_CACHE = {}


def run(cfg, inp, trace=False):
    from concourse import bass_utils

    maps, counts = prep(cfg, np.asarray(inp["x"], np.float32), inp["edge_index"], inp["batch"])
    w = prep_weights(cfg, inp)
    for m in maps:
        m.update(w)

    key = (cfg["N"], cfg["G"], tuple(tuple(t) for t in cfg["TS"]))
    if key not in _BUILD_CACHE:
        _BUILD_CACHE[key] = build(cfg)
    nc = _BUILD_CACHE[key]

    res = bass_utils.run_bass_kernel_spmd(
        nc, maps, core_ids=list(range(NCORES)), trace=trace
    )
    total = np.zeros((cfg["G"], OUT_CH), np.float64)
    for k in range(NCORES):
        total += res.results[k]["pool_out"].astype(np.float64)
    out = (total / np.maximum(counts, 1.0)[:, None]).astype(np.float32)
    return out, res


def kernel(**inputs) -> np.ndarray:
    cfg = make_cfg()
    out, _ = run(cfg, inputs, trace=False)
    return out


# revision 43
# speedup vs baseline: 1.0419x; 1.0280x over previous
"""GATv2 3-layer GNN on 8 Trainium2 NeuronCores (Bass/Tile) — v5.

Key structure (per core):
  - Nodes are host-binned into 8*49=392 blocks of 128 slots with balanced
    in-degree per block.
  - The xl gather uses gpsimd dma_gather (one instruction per table-half
    per block, ~1us SWDGE each) instead of per-tile indirect DMAs
    (994ns fixed overhead each, 16 per block).  dma_gather indices are
    int16, so the 50176-row table is split in two halves; each block's
    edges are packed half-0-first into whole 128-slot tiles, padded with
    dummy row-0 gathers so every core runs identical shapes (SPMD).
  - Tables are bf16 (dma_gather needs 256B-multiple rows; also improves
    accuracy over fp8).  Layer 2 (D=64) pads table rows to 128 cols.
  - Layer 0: every core builds the FULL xl0 table locally from a
    pre-transposed bf16 copy of x (no AllGather for layer 0).
  - Layers 1,2: xl shards are exchanged with a 2-chunk AllGather
    (blocks 0..CHA-1 early, rest late) so most of the exchange hides
    behind phase-B work of the producing layer.
  - Phase B is software-pipelined: stage1 (mask build, xr-expand
    matmuls batched into one PSUM tile + one copy per half, dma_gather)
    runs LAG blocks ahead of stage2 (edge math, one-hot aggregation,
    node update + fused projection of the NEXT layer).
  - bias trick: the table holds x@Wl WITHOUT bias; bl is folded into
    xr's bias (v = xl'+xr' is unchanged) and into the output bias.
"""

import sys

if "/opt/trn_rl_repo" not in sys.path:
    sys.path.insert(0, "/opt/trn_rl_repo")

import numpy as np
import ml_dtypes

BF16 = ml_dtypes.bfloat16

NEG_SLOPE = 0.2
N_NODES = 50000
N_EDGES = 800000
N_GRAPHS = 64
IN_CH = 128
HIDDEN = 128
HEADS = 4
OUT_CH = 64
NCORES = 8


def make_cfg(n_nodes=N_NODES, n_graphs=N_GRAPHS, in_ch=IN_CH):
    npc = n_nodes // NCORES
    assert npc * NCORES == n_nodes
    nblk = (npc + 127) // 128
    np_pad = nblk * 128
    cha = max(1, (nblk * 4) // 5)  # early AG chunk (blocks [0, cha))
    trows = NCORES * np_pad
    return dict(
        N=n_nodes,
        G=n_graphs,
        NPC=npc,
        NP=np_pad,
        NBLK=nblk,
        CHA=cha,
        CHB=nblk - cha,
        GBLK=NCORES * nblk,
        TROWS=trows,
        HB1=16768,
        HB2=33536,
        IN_CH=in_ch,
        TS=None,  # per-block tile counts per table third (list of 3 lists)
        LAYERS=[
            (in_ch, HIDDEN, HEADS, HIDDEN // HEADS, True),
            (HIDDEN, HIDDEN, HEADS, HIDDEN // HEADS, True),
            (HIDDEN, OUT_CH, 1, OUT_CH, False),
        ],
    )


# ---------------------------------------------------------------- host prep
def _balanced_bins(deg, nbins, binsz):
    """Assign nodes to bins (each bin holds exactly binsz nodes) minimizing
    max total degree per bin.  Greedy: degree-desc, min-load non-full bin.
    Returns slot_of[node] = bin*binsz + position."""
    import heapq

    n = deg.shape[0]
    order = np.argsort(-deg, kind="stable")
    heap = [(0, b) for b in range(nbins)]
    heapq.heapify(heap)
    fill = np.zeros(nbins, np.int64)
    load = np.zeros(nbins, np.int64)
    slot_of = np.empty(n, np.int64)
    for nd in order:
        while True:
            l, b = heapq.heappop(heap)
            if fill[b] < binsz:
                break
        slot_of[nd] = b * binsz + fill[b]
        fill[b] += 1
        load[b] += deg[nd]
        if fill[b] < binsz:
            heapq.heappush(heap, (load[b], b))
    return slot_of, int(load.max())


def tabrow_of_slot(cfg, slot):
    """Map global slot id -> table row (2-chunk AllGather layout)."""
    NP, NBLK, CHA = cfg["NP"], cfg["NBLK"], cfg["CHA"]
    c = slot // NP
    loc = slot % NP
    b = loc // 128
    r = loc % 128
    rowsA = NCORES * CHA * 128
    return np.where(
        b < CHA,
        c * CHA * 128 + b * 128 + r,
        rowsA + c * (NBLK - CHA) * 128 + (b - CHA) * 128 + r,
    )


def _wrap16(lst):
    """dma_gather index layout: idx k -> [k%16, k//16], replicated x8."""
    n = lst.shape[0]
    assert n % 16 == 0
    w = lst.reshape(n // 16, 16).T  # [16, W]
    return np.tile(w, (8, 1)).astype(np.int16)  # [128, W]


def prep(cfg, x, edge_index, batch):
    NPC, NP, NBLK, G, CHA = cfg["NPC"], cfg["NP"], cfg["NBLK"], cfg["G"], cfg["CHA"]
    GBLK = cfg["GBLK"]
    Din = cfg["IN_CH"]
    src = np.asarray(edge_index[0], dtype=np.int64)
    dst = np.asarray(edge_index[1], dtype=np.int64)
    batch = np.asarray(batch, dtype=np.int64)
    x = np.asarray(x, dtype=np.float32)
    N = x.shape[0]

    deg = np.bincount(dst, minlength=N)
    slot_of, maxload = _balanced_bins(deg, GBLK, 128)

    node_of_slot = np.full(GBLK * 128, -1, np.int64)
    node_of_slot[slot_of] = np.arange(N)

    # permuted x, laid out in TABLE-ROW block order, transposed per block
    x_slot = np.zeros((GBLK * 128, Din), np.float32)
    valid = node_of_slot >= 0
    x_slot[valid] = x[node_of_slot[valid]]
    tabrow = np.asarray(tabrow_of_slot(cfg, np.arange(GBLK * 128)))
    x_tab = np.zeros_like(x_slot)
    x_tab[tabrow] = x_slot
    assert GBLK % 8 == 0
    xfullT = (
        x_tab.reshape(GBLK // 8, 8, 128, Din)
        .transpose(0, 3, 1, 2)
        .reshape((GBLK // 8) * Din, 8 * 128)
    ).astype(BF16)

    # edges
    sd = slot_of[dst]
    ss = slot_of[src]
    trow = np.asarray(tabrow_of_slot(cfg, ss))
    core_of = sd // NP
    dloc = sd % NP
    bloc = dloc // 128
    drow = dloc % 128

    # ---- pass 1: per (core, block) edge lists split by table third
    HB = [0, cfg["HB1"], cfg["HB2"], cfg["TROWS"]]
    third = np.digitize(trow, HB[1:3]).astype(np.int64)
    key = (core_of * NBLK + bloc) * 3 + third
    order = np.argsort(key, kind="stable")
    ks = key[order]
    tr_s = trow[order]
    ed_s = drow[order]
    bounds = np.searchsorted(ks, np.arange(NCORES * NBLK * 3 + 1))
    ed_rows = [[None] * NBLK for _ in range(NCORES)]  # [(rows_i, d_i) x3]
    for c in range(NCORES):
        for b in range(NBLK):
            k0i = (c * NBLK + b) * 3
            ed_rows[c][b] = [
                (tr_s[bounds[k0i + i] : bounds[k0i + i + 1]] - HB[i],
                 ed_s[bounds[k0i + i] : bounds[k0i + i + 1]])
                for i in range(3)]

    # ---- pass 2: shared per-block tile counts (max over cores)
    TS = [[1] * NBLK for _ in range(3)]
    for b in range(NBLK):
        for c in range(NCORES):
            for i in range(3):
                r_i, _ = ed_rows[c][b][i]
                TS[i][b] = max(TS[i][b], (len(r_i) + 127) // 128)
    assert max(max(t) for t in TS) <= 8, [max(t) for t in TS]
    cfg["TS"] = TS
    TMAXH = [max(t) for t in TS]
    TMAX = max(a + b + c_ for a, b, c_ in zip(*TS))
    cfg["TMAXH"], cfg["TMAX"] = TMAXH, TMAX
    WS = [t * 8 for t in TMAXH]

    # ---- pass 3: per-core arrays
    maps = []
    F8 = ml_dtypes.float8_e4m3
    ar128 = np.arange(128, dtype=np.int64)
    WTOT = sum(WS)
    for c in range(NCORES):
        idxg = np.zeros((NBLK * 128, WTOT), np.int16)
        mT_h = np.zeros((NBLK * 128, TMAX * 128), F8)
        mE_h = np.zeros((NBLK * 128, TMAX * 128), F8)
        for b in range(NBLK):
            ts = [TS[i][b] for i in range(3)]
            Tb = sum(ts)
            dv = np.full(Tb * 128, -1, np.int64)
            woff, soff = 0, 0
            for i in range(3):
                r_i, d_i = ed_rows[c][b][i]
                Li = np.zeros(ts[i] * 128, np.int64)
                Li[: len(r_i)] = r_i
                idxg[b * 128 : (b + 1) * 128, woff : woff + ts[i] * 8] = _wrap16(Li)
                dv[soff : soff + len(d_i)] = d_i
                woff += WS[i]
                soff += ts[i] * 128
            mT_h[b * 128 : (b + 1) * 128, : Tb * 128] = (
                dv[None, :] == ar128[:, None]).astype(F8)
            dc = dv.reshape(Tb, 128).T  # [128(p), Tb]
            mE_h[b * 128 : (b + 1) * 128, : Tb * 128] = (
                dc[:, :, None] == ar128[None, None, :]).reshape(128, Tb * 128).astype(F8)

        # own x^T blocks (for the xr projection pass), in own-block order
        own_tabrows = np.asarray(tabrow_of_slot(cfg, c * NP + np.arange(NP)))
        xownT = (
            x_tab[own_tabrows]
            .reshape(NBLK, 128, Din)
            .transpose(0, 2, 1)
            .reshape(NBLK * Din, 128)
        ).astype(BF16)

        # pool mask [NP, G] over own slots
        pm = np.zeros((NP, G), np.float32)
        own_nodes = node_of_slot[c * NP : (c + 1) * NP]
        vv = own_nodes >= 0
        pm[np.arange(NP)[vv], batch[own_nodes[vv]]] = 1.0

        maps.append(
            dict(
                xfullT=xfullT,
                xownT=xownT,
                idxg=idxg,
                mT_h=mT_h,
                mE_h=mE_h,
                pool_mask=pm.astype(BF16),
            )
        )

    counts = np.bincount(batch, minlength=G).astype(np.float32)
    return maps, counts


def prep_weights(cfg, inp):
    w = {}
    for l in range(3):
        Wl = np.asarray(inp[f"Wl{l}"], np.float32)
        bl = np.asarray(inp[f"bl{l}"], np.float32)
        Wr = np.asarray(inp[f"Wr{l}"], np.float32)
        br = np.asarray(inp[f"br{l}"], np.float32)
        bo = np.asarray(inp[f"bias{l}"], np.float32)
        D = Wl.shape[1]
        # table holds x@Wl (no bias); xr bias = bl+br; out bias += bl
        w[f"wcat{l}"] = np.concatenate([Wl, Wr], axis=1).astype(BF16)  # [Din,2D]
        w[f"bias_r{l}"] = np.broadcast_to((bl + br)[None, :], (128, D)).copy()
        w[f"bias_out{l}"] = np.broadcast_to((bo + bl)[None, :], (128, D)).copy()
    TMAX = cfg["TMAX"]
    for l in range(3):
        D = [HIDDEN, HIDDEN, OUT_CH][l]
        w[f"att{l}r"] = np.broadcast_to(
            np.asarray(inp[f"att{l}"], np.float32).reshape(1, 1, D), (128, TMAX, D)
        ).reshape(128, TMAX * D).astype(BF16)
    w["ident"] = np.eye(128, dtype=np.float32)
    return w


# ---------------------------------------------------------------- device build
def build(cfg):
    from concourse import bass, bacc, mybir
    import concourse.tile as tile
    from concourse.tile import add_dep_helper

    F32 = mybir.dt.float32
    BF = mybir.dt.bfloat16
    F8 = mybir.dt.float8e4
    I16 = mybir.dt.int16
    A = mybir.AluOpType
    ACTF = mybir.ActivationFunctionType

    NP, NBLK, TROWS, G = cfg["NP"], cfg["NBLK"], cfg["TROWS"], cfg["G"]
    CHA, CHB, GBLK = cfg["CHA"], cfg["CHB"], cfg["GBLK"]
    HB = [0, cfg["HB1"], cfg["HB2"], cfg["TROWS"]]
    TS, TMAX = cfg["TS"], cfg["TMAX"]
    TMAXH = cfg["TMAXH"]
    WS = [t * 8 for t in TMAXH]
    WTOT = sum(WS)
    WOFFS = [0, WS[0], WS[0] + WS[1]]
    Din0 = cfg["IN_CH"]
    LAYERS = cfg["LAYERS"]
    LAG = 2

    nc = bacc.Bacc(
        "TRN2",
        target_bir_lowering=False,
        debug=False,
        enable_asserts=False,
        num_devices=NCORES,
        num_swdge_queues=4,
    )

    ext = {}

    def ein(name, shape, dt):
        ext[name] = nc.dram_tensor(name, shape, dt, kind="ExternalInput").ap()
        return ext[name]

    xfullT = ein("xfullT", [(GBLK // 8) * Din0, 8 * 128], BF)
    xownT = ein("xownT", [NBLK * Din0, 128], BF)
    idxg_d = ein("idxg", [NBLK * 128, WTOT], I16)
    mT_d = ein("mT_h", [NBLK * 128, TMAX * 128], F8)
    mE_d = ein("mE_h", [NBLK * 128, TMAX * 128], F8)
    pool_mask = ein("pool_mask", [NP, G], BF)
    ident_d = ein("ident", [128, 128], F32)
    wcat_d, biasr_d, att_d, biasout_d = [], [], [], []
    for l, (Din, D, H, C, _) in enumerate(LAYERS):
        wcat_d.append(ein(f"wcat{l}", [Din, 2 * D], BF))
        biasr_d.append(ein(f"bias_r{l}", [128, D], F32))
        att_d.append(ein(f"att{l}r", [128, TMAX * D], BF))
        biasout_d.append(ein(f"bias_out{l}", [128, D], F32))

    pool_out = nc.dram_tensor("pool_out", [G, OUT_CH], F32, kind="ExternalOutput").ap()

    # internal DRAM: tables are bf16, 128 cols even for layer 2 (gather rows
    # must be 256B multiples)
    tabs = []
    ccA, ccB = [None] * 3, [None] * 3
    for l in range(3):
        tabs.append(
            nc.dram_tensor(
                f"tab{l}", [TROWS, 128], BF, kind="Internal", addr_space="Shared"
            ).ap()
        )
        if l >= 1:
            ccA[l] = nc.dram_tensor(f"ccA{l}", [CHA * 128, 128], BF, kind="Internal").ap()
            ccB[l] = nc.dram_tensor(f"ccB{l}", [CHB * 128, 128], BF, kind="Internal").ap()

    from contextlib import ExitStack

    with tile.TileContext(nc) as tc, ExitStack() as pools:
        const = pools.enter_context(tc.tile_pool(name="const", bufs=1))
        s1 = pools.enter_context(tc.tile_pool(name="s1", bufs=LAG + 2))
        s1b = pools.enter_context(tc.tile_pool(name="s1b", bufs=3))
        s2 = pools.enter_context(tc.tile_pool(name="s2", bufs=3))
        nodep = pools.enter_context(tc.tile_pool(name="nodep", bufs=3))
        # PSUM: 8 banks x 2KB.  vexp 4 banks, agg 1, pam 1, pat 1, pool 1.
        psum_exp = pools.enter_context(tc.tile_pool(name="psum_exp", bufs=1, space="PSUM"))
        psum_agg = pools.enter_context(tc.tile_pool(name="psum_agg", bufs=2, space="PSUM"))
        psum_pam = pools.enter_context(tc.tile_pool(name="psum_pam", bufs=2, space="PSUM"))
        psum_pat = pools.enter_context(tc.tile_pool(name="psum_pat", bufs=1, space="PSUM"))
        psum_pool = pools.enter_context(tc.tile_pool(name="psum_pool", bufs=1, space="PSUM"))

        # persistent SBUF: xr tables (double-buffered across layers)
        xr_sb = [
            nc.alloc_sbuf_tensor(f"xr_sb{k}", [128, NBLK, HIDDEN], BF).ap()
            for k in range(2)
        ]

        def const_tile(shape, dt, src_ap, tag):
            t = const.tile(shape, dt, tag=tag)
            nc.sync.dma_start(out=t[:], in_=src_ap)
            return t

        ident = const_tile([128, 128], F32, ident_d[:], "ident")
        wcat_s, biasr_s, att_s, biasout_s = [], [], [], []
        for l, (Din, D, H, C, _) in enumerate(LAYERS):
            wcat_s.append(const_tile([Din, 2 * D], BF, wcat_d[l][:], f"wc{l}"))
            biasr_s.append(const_tile([128, D], F32, biasr_d[l][:], f"br{l}"))
            att_s.append(const_tile([128, TMAX * D], BF, att_d[l][:], f"at{l}"))
            biasout_s.append(const_tile([128, D], F32, biasout_d[l][:], f"bo{l}"))

        # ============ layer 0: local full-table build + own xr pass
        D0 = LAYERS[0][1]
        tab0_writes = []
        GRP = 8
        assert GBLK % GRP == 0
        for gg in range(GBLK // GRP):
            xT8 = nodep.tile([Din0, GRP, 128], BF, tag="t0_xT")
            nc.sync.dma_start(
                out=xT8[:],
                in_=xfullT[gg * Din0 : (gg + 1) * Din0, :],
            )
            vps8 = psum_exp.tile([128, GRP, D0], F32, tag="vexp")
            for k in range(GRP):
                nc.tensor.matmul(
                    out=vps8[:, k, :], lhsT=xT8[:, k, :], rhs=wcat_s[0][:, :D0],
                    start=True, stop=True
                )
            xl8 = nodep.tile([128, GRP, D0], BF, tag="t0_xl")
            nc.scalar.copy(out=xl8[:], in_=vps8[:])
            wi = nc.sync.dma_start(
                out=tabs[0][gg * GRP * 128 : (gg + 1) * GRP * 128, :].rearrange(
                    "(g p) d -> p g d", g=GRP
                ),
                in_=xl8[:],
            )
            tab0_writes.append(wi)

        for b in range(NBLK):
            xT = nodep.tile([Din0, 128], BF, tag="own_xT")
            nc.sync.dma_start(out=xT[:], in_=xownT[b * Din0 : (b + 1) * Din0, :])
            pr_f = psum_pam.tile([128, 2 * HIDDEN], F32, tag="pa_mm")
            pr = pr_f[:, :D0]
            nc.tensor.matmul(
                out=pr, lhsT=xT[:], rhs=wcat_s[0][:, D0:], start=True, stop=True
            )
            nc.vector.tensor_tensor(
                out=xr_sb[0][:, b, :D0], in0=pr, in1=biasr_s[0][:], op=A.add
            )

        # per-third barrier proxies: third-h gathers only need table rows
        # [HB[h], HB[h+1]), i.e. the build groups covering those rows
        GROWS = GRP * 128
        barriers0 = []
        for h in range(3):
            g_lo = HB[h] // GROWS
            g_hi = (HB[h + 1] + GROWS - 1) // GROWS
            bar = nc.scalar.copy(out=ident[:1, h : h + 1], in_=ident[:1, h : h + 1])
            for wi in tab0_writes[g_lo:g_hi]:
                add_dep_helper(bar.ins, wi.ins, sync=True, reason=f"tab0 third{h}")
            barriers0.append(bar)

        # ============ layers
        ag_calls = {0: barriers0}  # per-layer: dep list (len 3 => per-third)

        for l, (Din, D, H, C, use_elu) in enumerate(LAYERS):
            HD = H + D
            xr_cur = xr_sb[l % 2]
            xr_nxt = xr_sb[(l + 1) % 2]
            gather_deps = ag_calls[l]
            if l < 2:
                pa_writesA, pa_writesB = [], []
            if l == 2:
                pool_ps = psum_pool.tile([G, OUT_CH], F32, tag="pool")

            state = {}

            def stage1(b, l=l, D=D, state=state,
                       xr_cur=xr_cur, gather_deps=gather_deps):
                ts = [TS[i][b] for i in range(3)]
                Tb = sum(ts)
                toffs = [0, ts[0], ts[0] + ts[1]]
                idxt = s1.tile([128, WTOT], I16, tag="idxt")
                nc.sync.dma_start(
                    out=idxt[:], in_=idxg_d[b * 128 : (b + 1) * 128, :]
                )
                # batched gathers FIRST (long DMA drain overlaps the rest of
                # stage1): dma_gather per table third, chunked to <=8 tiles
                # (1024 descs) -- the SWDGE ring holds 1024 descriptors
                g_all = s1.tile([128, Tb, 128], BF, tag="g")
                qn = b % 4
                for h in range(3):
                    tb_h, toff, ioff = ts[h], toffs[h], WOFFS[h]
                    roff, rend = HB[h], HB[h + 1]
                    deps_h = (
                        [gather_deps[h]] if len(gather_deps) == 3 else gather_deps)
                    done = 0
                    while done < tb_h:
                        ch = min(8, tb_h - done)
                        gi = nc.gpsimd.dma_gather(
                            out_ap=g_all[:, toff + done : toff + done + ch, :],
                            in_ap=tabs[l][roff:rend, :],
                            idxs_ap=idxt[:, ioff + done * 8 : ioff + (done + ch) * 8],
                            num_idxs=ch * 128, num_idxs_reg=ch * 128,
                            elem_size=128, queue_num=qn)
                        for dep in deps_h:
                            add_dep_helper(gi.ins, dep.ins, sync=True,
                                           reason="gather after table ready")
                        done += ch
                        qn = (qn + 1) % 4
                # host-precomputed one-hot masks (fp8): mT for xr-expansion,
                # mE for the dst aggregation
                mT = s1b.tile([128, Tb, 128], F8, tag="mT")
                nc.sync.dma_start(
                    out=mT[:].rearrange("p t q -> p (t q)"),
                    in_=mT_d[b * 128 : (b + 1) * 128, : Tb * 128])
                mE = s1.tile([128, Tb, 128], F8, tag="mE")
                nc.sync.dma_start(
                    out=mE[:].rearrange("p t q -> p (t q)"),
                    in_=mE_d[b * 128 : (b + 1) * 128, : Tb * 128])
                # xr expansion per third into one PSUM tile + one copy
                v_all = s1.tile([128, Tb, D], BF, tag="v")
                for h in range(3):
                    tb, toff = ts[h], toffs[h]
                    vps = psum_exp.tile([128, max(max(TMAXH), GRP), D0], F32,
                                        tag="vexp")
                    for t in range(tb):
                        nc.tensor.matmul(
                            out=vps[:, t, :D], lhsT=mT[:, toff + t, :],
                            rhs=xr_cur[:, b, :D], start=True, stop=True)
                    nc.scalar.copy(
                        out=v_all[:, toff : toff + tb, :], in_=vps[:, :tb, :D])
                state[b] = (v_all, g_all, mE)

            def stage2(b, l=l, D=D, H=H, C=C, HD=HD, state=state,
                       xr_cur=xr_cur, xr_nxt=xr_nxt, use_elu=use_elu):
                Tb = sum(TS[i][b] for i in range(3))
                v_all, g_all, mE = state.pop(b)
                gD = g_all[:, :, :D]
                vs = s2.tile([128, Tb, D], BF, tag="vsum")
                nc.vector.tensor_tensor(
                    out=vs[:], in0=gD, in1=v_all[:], op=A.add)
                v4 = gD.rearrange("p t (h c) -> p t h c", h=H)
                vf = vs[:].rearrange("p t d -> p (t d)")
                l_all = s2.tile([128, Tb * D], BF, tag="lrelu")
                nc.vector.scalar_tensor_tensor(
                    out=l_all[:], in0=vf, scalar=NEG_SLOPE, in1=vf,
                    op0=A.mult, op1=A.max)
                p_all = s2.tile([128, Tb, H, C], BF, tag="patt")
                nc.vector.tensor_tensor(
                    out=p_all[:].rearrange("p t h c -> p (t h c)"), in0=l_all[:],
                    in1=att_s[l][:, : Tb * D], op=A.mult)
                lg = s2.tile([128, Tb, H], F32, tag="lg")
                nc.vector.tensor_reduce(
                    out=lg[:], in_=p_all[:], axis=mybir.AxisListType.X, op=A.add)
                w_all = s2.tile([128, Tb, HD], BF, tag="wall")
                e_v = w_all[:, :, :H]
                nc.scalar.activation(out=e_v, in_=lg[:], func=ACTF.Exp)
                nc.vector.tensor_tensor(
                    out=w_all[:, :, H:].rearrange("p t (h c) -> p t h c", h=H),
                    in0=v4,
                    in1=e_v.rearrange("p t (h o) -> p t h o", o=1).to_broadcast(
                        [128, Tb, H, C]), op=A.mult)

                o_ps = psum_agg.tile([128, HD], F32, tag="agg")
                for t in range(Tb):
                    nc.tensor.matmul(
                        out=o_ps[:], lhsT=mE[:, t, :], rhs=w_all[:, t, :],
                        start=(t == 0), stop=(t == Tb - 1))

                dn = nodep.tile([128, H], F32, tag="dn")
                nc.vector.tensor_scalar(
                    out=dn[:], in0=o_ps[:, :H], scalar1=1e-30, scalar2=None, op0=A.add)
                rc = nodep.tile([128, H], F32, tag="rc")
                nc.vector.reciprocal(out=rc[:], in_=dn[:])
                onorm = nodep.tile([128, H, C], F32, tag="onorm")
                nc.vector.tensor_tensor(
                    out=onorm[:],
                    in0=o_ps[:, H:].rearrange("p (h c) -> p h c", h=H),
                    in1=rc[:].rearrange("p (h o) -> p h o", h=H).to_broadcast([128, H, C]),
                    op=A.mult)
                hb = nodep.tile([128, D], F32 if use_elu else BF,
                                tag="hb" if use_elu else "hbb")
                nc.vector.tensor_tensor(
                    out=hb[:], in0=onorm[:].rearrange("p h c -> p (h c)"),
                    in1=biasout_s[l][:], op=A.add)
                if use_elu:
                    amax = nodep.tile([128, D], F32, tag="amax")
                    nc.vector.tensor_scalar(
                        out=amax[:], in0=hb[:], scalar1=0.0, scalar2=None, op0=A.max)
                    amin = nodep.tile([128, D], F32, tag="amin")
                    nc.vector.tensor_scalar(
                        out=amin[:], in0=hb[:], scalar1=0.0, scalar2=None, op0=A.min)
                    aexp = nodep.tile([128, D], F32, tag="aexp")
                    nc.scalar.activation(out=aexp[:], in_=amin[:], func=ACTF.Exp)
                    h_t = nodep.tile([128, D], F32, tag="h")
                    nc.vector.scalar_tensor_tensor(
                        out=h_t[:], in0=amax[:], scalar=-1.0, in1=aexp[:],
                        op0=A.add, op1=A.add)
                    # fused projection for layer l+1
                    Dn = LAYERS[l + 1][1]
                    tp = psum_pat.tile([128, 128], F32, tag="pa_tr")
                    nc.tensor.transpose(out=tp[:], in_=h_t[:], identity=ident[:])
                    hT = nodep.tile([128, 128], BF, tag="pa_hT")
                    nc.scalar.copy(out=hT[:], in_=tp[:])
                    pp_f = psum_pam.tile([128, 2 * HIDDEN], F32, tag="pa_mm")
                    pp = pp_f[:, : 2 * Dn]
                    nc.tensor.matmul(
                        out=pp, lhsT=hT[:], rhs=wcat_s[l + 1][:],
                        start=True, stop=True)
                    xl_t = nodep.tile([128, 128], BF, tag="pa_xl")
                    nc.scalar.copy(out=xl_t[:, :Dn], in_=pp[:, :Dn])
                    if Dn < 128:
                        nc.scalar.copy(out=xl_t[:, Dn:], in_=pp[:, :128 - Dn])
                    if b < CHA:
                        wi = nc.sync.dma_start(
                            out=ccA[l + 1][b * 128 : (b + 1) * 128, :], in_=xl_t[:])
                        pa_writesA.append(wi)
                    else:
                        wi = nc.sync.dma_start(
                            out=ccB[l + 1][(b - CHA) * 128 : (b - CHA + 1) * 128, :],
                            in_=xl_t[:])
                        pa_writesB.append(wi)
                    nc.vector.tensor_tensor(
                        out=xr_nxt[:, b, :Dn], in0=pp[:, Dn : 2 * Dn],
                        in1=biasr_s[l + 1][:], op=A.add)
                    if b == CHA - 1:
                        agA = nc.gpsimd.collective_compute(
                            "AllGather", A.bypass,
                            replica_groups=[list(range(NCORES))],
                            ins=[ccA[l + 1][:]],
                            outs=[tabs[l + 1][0 : NCORES * CHA * 128, :]],
                        )
                        for wi in pa_writesA:
                            add_dep_helper(agA.ins, wi.ins, sync=True, reason="agA")
                        ag_calls[l + 1] = [agA]
                    if b == NBLK - 1:
                        agB = nc.gpsimd.collective_compute(
                            "AllGather", A.bypass,
                            replica_groups=[list(range(NCORES))],
                            ins=[ccB[l + 1][:]],
                            outs=[tabs[l + 1][NCORES * CHA * 128 :, :]],
                        )
                        for wi in pa_writesB:
                            add_dep_helper(agB.ins, wi.ins, sync=True, reason="agB")
                        ag_calls[l + 1].append(agB)
                else:
                    pm_t = nodep.tile([128, G], BF, tag="pmt")
                    nc.sync.dma_start(
                        out=pm_t[:], in_=pool_mask[b * 128 : (b + 1) * 128, :])
                    nc.tensor.matmul(
                        out=pool_ps[:], lhsT=pm_t[:], rhs=hb[:],
                        start=(b == 0), stop=(b == NBLK - 1))

            for b in range(NBLK + LAG):
                if b < NBLK:
                    stage1(b)
                if b >= LAG:
                    stage2(b - LAG)

        pool_sb = nodep.tile([G, OUT_CH], F32, tag="poolsb")
        nc.scalar.copy(out=pool_sb[:], in_=pool_ps[:])
        nc.sync.dma_start(out=pool_out[:], in_=pool_sb[:])

    nc.compile()
    return nc


# ---------------------------------------------------------------- runner
_BUILD_CACHE = {}


def run(cfg, inp, trace=False):
    from concourse import bass_utils

    maps, counts = prep(cfg, np.asarray(inp["x"], np.float32), inp["edge_index"], inp["batch"])
    w = prep_weights(cfg, inp)
    for m in maps:
        m.update(w)

    key = (cfg["N"], cfg["G"], tuple(tuple(t) for t in cfg["TS"]))
    if key not in _BUILD_CACHE:
        _BUILD_CACHE[key] = build(cfg)
    nc = _BUILD_CACHE[key]

    res = bass_utils.run_bass_kernel_spmd(
        nc, maps, core_ids=list(range(NCORES)), trace=trace
    )
    total = np.zeros((cfg["G"], OUT_CH), np.float64)
    for k in range(NCORES):
        total += res.results[k]["pool_out"].astype(np.float64)
    out = (total / np.maximum(counts, 1.0)[:, None]).astype(np.float32)
    return out, res


def kernel(**inputs) -> np.ndarray:
    cfg = make_cfg()
    out, _ = run(cfg, inputs, trace=False)
    return out
